# revision 17
# baseline (speedup 1.0000x reference)
"""AdaptiveGRU (2-layer, H=1024, adaptive reset gate) on 8 Trainium2 NeuronCores.

Strategy: tensor-parallel over the 3H gate dim (each core owns a 128-row
h-slice of both layers), fp32 everywhere.
 - Layer-0 input projections (x @ W_ih0.T + biases) are precomputed for all
   timesteps as one large GEMM (x distributed to all cores via a startup
   AllGather of host-transposed shards).
 - The BinaryGate is folded: g = sigmoid(x_{t+1}.v1 + out_t.v2 + c) with
   v1 = (gWF@gW1), v2 = (gWF@gW2), c = gWF.(gb1+gb2)+gbF.
 - Per step: 2 AllGathers (a_raw slices; b_raw slices + partial gate dots).
 - Reset/freeze masking is applied on matmul OUTPUTS (select-on-output):
   gh_used = select(reset -> W@h0, invalid -> prev used, else W@state_raw),
   which keeps masking off the matmul critical path.
"""
import os
import sys

sys.path.insert(0, '/opt/trn_rl_repo')

import numpy as np

import concourse.bass as bass
import concourse.bacc as bacc
import concourse.mybir as mybir
from concourse import tile
from concourse.bass_utils import run_bass_kernel_spmd

H = 1024
B = 64
NC = 8
P = 128
SH = 3 * H // NC       # 384 rows of each gate matrix per core
M3 = 3                 # m-tiles (r, z, n) per core
K8 = H // P            # 8 contraction tiles
dt = mybir.dt.float32

LAST_EXEC_NS = [None]

_prog_cache = {}


def _build(T):
    TB = T * B
    TCH = TB // NC                       # per-core xT chunk cols
    CHUNK = min(512, TCH)                # precompute N-chunk size
    NCH = TB // CHUNK
    CH_PER_BLK = TCH // CHUNK            # chunks per AG block

    nc = bacc.Bacc("TRN2", target_bir_lowering=False, debug=False, num_devices=NC)

    # ---------------- DRAM I/O ----------------
    xT_d = nc.dram_tensor("xT", [H, TCH], dt, kind="ExternalInput")
    wih0_d = nc.dram_tensor("wih0", [H, SH], dt, kind="ExternalInput")
    whh0_d = nc.dram_tensor("whh0", [H, SH], dt, kind="ExternalInput")
    wih1_d = nc.dram_tensor("wih1", [H, SH], dt, kind="ExternalInput")
    whh1_d = nc.dram_tensor("whh1", [H, SH], dt, kind="ExternalInput")
    bsum0_d = nc.dram_tensor("bsum0", [P, 3], dt, kind="ExternalInput")
    bsum1_d = nc.dram_tensor("bsum1", [P, 3], dt, kind="ExternalInput")
    v1_d = nc.dram_tensor("v1", [P, K8], dt, kind="ExternalInput")
    v2s_d = nc.dram_tensor("v2s", [P, 1], dt, kind="ExternalInput")
    h0T_d = nc.dram_tensor("h0T", [P, K8], dt, kind="ExternalInput")
    h0c_d = nc.dram_tensor("h0c", [P, 1], dt, kind="ExternalInput")
    validc_d = nc.dram_tensor("validc", [B, T], dt, kind="ExternalInput")
    invalidc_d = nc.dram_tensor("invalidc", [B, T], dt, kind="ExternalInput")
    islastc_d = nc.dram_tensor("islastc", [B, T], dt, kind="ExternalInput")
    id64_d = nc.dram_tensor("id64", [B, B], dt, kind="ExternalInput")
    ones1_d = nc.dram_tensor("ones1", [1, P], dt, kind="ExternalInput")
    c0_d = nc.dram_tensor("c0", [B, 1], dt, kind="ExternalInput")
    bhn_d = nc.dram_tensor("bhn", [P, 2], dt, kind="ExternalInput")

    out_d = nc.dram_tensor("out", [T, P, B], dt, kind="ExternalOutput")
    gates_d = nc.dram_tensor("gates", [B, T], dt, kind="ExternalOutput")
    DBG = os.environ.get("KERNEL_DEBUG", "0") == "1"
    if DBG:
        dbg_gi0 = nc.dram_tensor("dbg_gi0", [P, M3 * B], dt, kind="ExternalOutput")
        dbg_araw = nc.dram_tensor("dbg_araw", [P, B], dt, kind="ExternalOutput")
        dbg_agA = nc.dram_tensor("dbg_agA", [P, NC * B], dt, kind="ExternalOutput")
        dbg_psA = nc.dram_tensor("dbg_psA", [P, M3 * B], dt, kind="ExternalOutput")
        dbg_braw = nc.dram_tensor("dbg_braw", [P, B], dt, kind="ExternalOutput")
        dbg_sc = nc.dram_tensor("dbg_sc", [B, 1], dt, kind="ExternalOutput")
        dbg_g1x = nc.dram_tensor("dbg_g1x", [B, T], dt, kind="ExternalOutput")
        dbg_w = nc.dram_tensor("dbg_w", [P, 2 * SH], dt, kind="ExternalOutput")
        dbg_ev = nc.dram_tensor("dbg_ev", [P, 512], dt, kind="ExternalOutput")
        dbg_xt = nc.dram_tensor("dbg_xt", [P, 512], dt, kind="ExternalOutput")

    rg = [list(range(NC))]
    Sig = mybir.ActivationFunctionType.Sigmoid
    Tanh = mybir.ActivationFunctionType.Tanh
    Alu = mybir.AluOpType

    with tile.TileContext(nc) as tc:
        with (
            tc.tile_pool(name="const", bufs=1) as cp,
            tc.tile_pool(name="dram", bufs=1, space="DRAM") as dp,
        ):
            # ---------- resident constants ----------
            wih0_sb = cp.tile([P, K8 * SH], dt)
            whh0_sb = cp.tile([P, K8 * SH], dt)
            wih1_sb = cp.tile([P, K8 * SH], dt)
            whh1_sb = cp.tile([P, K8 * SH], dt)
            for w_sb, w_d in ((wih0_sb, wih0_d), (whh0_sb, whh0_d),
                              (wih1_sb, wih1_d), (whh1_sb, whh1_d)):
                nc.sync.dma_start(
                    w_sb.rearrange("p (k m) -> p k m", m=SH),
                    w_d.ap().rearrange("(k p) m -> p k m", p=P))

            bsum0_sb = cp.tile([P, 3], dt)
            bsum1_sb = cp.tile([P, 3], dt)
            v1_sb = cp.tile([P, K8], dt)
            v2s_sb = cp.tile([P, 1], dt)
            h0T_sb = cp.tile([P, K8], dt)
            h0c_sb = cp.tile([P, 1], dt)
            validc_sb = cp.tile([B, T], dt)
            invalidc_sb = cp.tile([B, T], dt)
            islastc_sb = cp.tile([B, T], dt)
            id64_sb = cp.tile([B, B], dt)
            ones1_sb = cp.tile([1, P], dt)
            c0_sb = cp.tile([B, 1], dt)
            bhn_sb = cp.tile([P, 2], dt)
            for sb, d in ((bsum0_sb, bsum0_d), (bsum1_sb, bsum1_d),
                          (v1_sb, v1_d), (v2s_sb, v2s_d), (h0T_sb, h0T_d),
                          (h0c_sb, h0c_d), (validc_sb, validc_d),
                          (invalidc_sb, invalidc_d), (islastc_sb, islastc_d),
                          (id64_sb, id64_d), (ones1_sb, ones1_d), (c0_sb, c0_d),
                          (bhn_sb, bhn_d)):
                nc.sync.dma_start(sb[:], d[:])

            ones_sb = cp.tile([P, B], dt)
            nc.vector.memset(ones_sb[:], 1.0)
            zeros_sb = cp.tile([P, B], dt)
            nc.vector.memset(zeros_sb[:], 0.0)
            h0bc = cp.tile([P, B], dt)
            nc.vector.tensor_scalar_mul(h0bc[:], ones_sb[:], h0c_sb[:, 0:1])

            ghh0bc = cp.tile([P, M3 * B], dt)   # broadcast W_hh0_shard @ h0
            ghh1bc = cp.tile([P, M3 * B], dt)
            G1x_sb = cp.tile([B, T], dt)        # x_{.}.v1 columns
            gates_sb = cp.tile([B, T], dt)

            # internal DRAM
            xt_int = dp.tile([H, TCH], dt, name="xt_int")
            xt_full = dp.tile([NC * H, TCH], dt, name="xt_full", addr_space="Shared")
            gi0_d = dp.tile([M3, P, TB], dt, name="gi0_d")

            # =======================================================
            # Phase 0: distribute xT
            # =======================================================
            nc.sync.dma_start(xt_int[:], xT_d[:])
            nc.gpsimd.collective_compute(
                "AllGather", Alu.bypass, replica_groups=rg,
                ins=[xt_int[:]], outs=[xt_full[:]])

            # =======================================================
            # Phase 1: precompute gi0 = x@Wih0_shard.T + bsum0, G1x = x.v1,
            #          ghh0/ghh1 consts
            # =======================================================
            with (
                tc.tile_pool(name="pre_sb", bufs=1) as pp,
                tc.tile_pool(name="pre_ps", bufs=1, space="PSUM") as pps,
            ):
                # ghh consts: W_shard @ h0  -> [P, 3] -> broadcast
                psh = pps.tile([P, 4], dt, tag="psh")
                ghc = pp.tile([P, 3], dt, tag="ghc")
                for w_sb, bc in ((whh0_sb, ghh0bc), (whh1_sb, ghh1bc)):
                    for m in range(M3):
                        for k in range(K8):
                            nc.tensor.matmul(
                                psh[:, m:m + 1],
                                w_sb[:, k * SH + m * P:k * SH + (m + 1) * P],
                                h0T_sb[:, k:k + 1],
                                start=(k == 0), stop=(k == K8 - 1))
                    nc.vector.tensor_copy(ghc[:], psh[:, 0:3])
                    for m in range(M3):
                        nc.vector.tensor_scalar_mul(
                            bc[:, m * B:(m + 1) * B], ones_sb[:], ghc[:, m:m + 1])

                for n in range(NCH):
                    blk, coff = divmod(n, CH_PER_BLK)
                    xt = pp.tile([P, K8 * CHUNK], dt, tag="xt", bufs=3)
                    nc.sync.dma_start(
                        xt.rearrange("p (k c) -> p k c", c=CHUNK),
                        xt_full[blk * H:(blk + 1) * H,
                                coff * CHUNK:(coff + 1) * CHUNK]
                        .rearrange("(k p) c -> p k c", p=P))
                    for m in range(M3):
                        ps = pps.tile([P, CHUNK], dt, tag="psPC", bufs=2)
                        for k in range(K8):
                            nc.tensor.matmul(
                                ps[:],
                                wih0_sb[:, k * SH + m * P:k * SH + (m + 1) * P],
                                xt[:, k * CHUNK:(k + 1) * CHUNK],
                                start=(k == 0), stop=(k == K8 - 1))
                        ev = pp.tile([P, CHUNK], dt, tag="ev", bufs=3)
                        nc.vector.tensor_scalar_add(ev[:], ps[:], bsum0_sb[:, m:m + 1])
                        if DBG and n == 0 and m == 0:
                            nc.sync.dma_start(dbg_ev[:, 0:CHUNK], ev[:])
                            nc.sync.dma_start(dbg_xt[:, 0:CHUNK], xt[:, 0:CHUNK])
                        nc.sync.dma_start(gi0_d[m, :, n * CHUNK:(n + 1) * CHUNK], ev[:])
                    for j in range(CHUNK // B):
                        tau = n * (CHUNK // B) + j
                        psg = pps.tile([B, 1], dt, tag="psg", bufs=2)
                        for k in range(K8):
                            nc.tensor.matmul(
                                psg[:],
                                xt[:, k * CHUNK + j * B:k * CHUNK + (j + 1) * B],
                                v1_sb[:, k:k + 1],
                                start=(k == 0), stop=(k == K8 - 1))
                        nc.vector.tensor_copy(G1x_sb[:, tau:tau + 1], psg[:])


            # =======================================================
            # Phase 2: recurrence
            # =======================================================
            with (
                tc.tile_pool(name="rec_sb", bufs=2) as rp,
                tc.tile_pool(name="rec_ps", bufs=1, space="PSUM") as rps,
                tc.tile_pool(name="rec_dram", bufs=2, space="DRAM") as rdp,
            ):
                psA = rps.tile([P, M3 * B], dt, tag="psA")
                psB = rps.tile([P, M3 * B], dt, tag="psB")
                psC = rps.tile([P, M3 * B], dt, tag="psC")
                psmr = rps.tile([P, M3 * B], dt, tag="psmr")   # reset bcast x3
                psmi = rps.tile([P, M3 * B], dt, tag="psmi")   # invalid bcast x3
                pssm = rps.tile([P, 260], dt, tag="pssm")      # pdot / rows3 / validbc

                gh0u_prev = None
                gh1u_prev = None
                mru8_p = None
                miu8_p = None
                aused_prev = None
                bused_prev = None
                agA_prev = None
                agB_prev = None

                for t in range(T):
                    # ---- layer-0 combine ----
                    g0t = rp.tile([P, M3 * B], dt, tag="g0t", bufs=3)
                    nc.scalar.dma_start(
                        g0t.rearrange("p (m c) -> p m c", c=B),
                        gi0_d[:, :, t * B:(t + 1) * B].rearrange("m p c -> p m c"))

                    if t == 0:
                        gh0u = ghh0bc
                        aprev = h0bc
                    else:
                        gh0u = rp.tile([P, M3 * B], dt, tag="gh0u")
                        nc.vector.tensor_copy(gh0u[:], psB[:])
                        nc.vector.copy_predicated(gh0u[:], mru8_p[:], ghh0bc[:])
                        nc.vector.copy_predicated(gh0u[:], miu8_p[:], gh0u_prev[:])
                        aprev = aused_prev
                    gh0u_prev = gh0u

                    t_r = rp.tile([P, B], dt, tag="t_r")
                    t_z = rp.tile([P, B], dt, tag="t_z")
                    t_n = rp.tile([P, B], dt, tag="t_n")
                    araw = rp.tile([P, B], dt, tag="araw")
                    nc.vector.tensor_add(t_r[:], g0t[:, 0:B], gh0u[:, 0:B])
                    nc.scalar.activation(t_r[:], t_r[:], Sig)
                    nc.vector.tensor_add(t_z[:], g0t[:, B:2 * B], gh0u[:, B:2 * B])
                    nc.scalar.activation(t_z[:], t_z[:], Sig)
                    nc.vector.tensor_scalar_add(t_n[:], gh0u[:, 2 * B:3 * B],
                                                bhn_sb[:, 0:1])
                    nc.vector.tensor_mul(t_n[:], t_r[:], t_n[:])
                    nc.vector.tensor_add(t_n[:], t_n[:], g0t[:, 2 * B:3 * B])
                    nc.scalar.activation(t_n[:], t_n[:], Tanh)
                    # a_raw = n + z*(aprev - n)
                    nc.vector.tensor_sub(araw[:], aprev[:], t_n[:])
                    nc.vector.tensor_mul(araw[:], araw[:], t_z[:])
                    nc.vector.tensor_add(araw[:], araw[:], t_n[:])

                    # ---- A exchange ----
                    agAin = rdp.tile([P, B], dt, tag="agAin")
                    agAout = rdp.tile([NC * P, B], dt, tag="agAout",
                                      addr_space="Shared")
                    nc.sync.dma_start(agAin[:], araw[:])
                    nc.gpsimd.collective_compute(
                        "AllGather", Alu.bypass, replica_groups=rg,
                        ins=[agAin[:]], outs=[agAout[:]])
                    agA = rp.tile([P, NC * B], dt, tag="agA")
                    nc.scalar.dma_start(
                        agA.rearrange("p (i b) -> p i b", b=B),
                        agAout.rearrange("(i p) b -> p i b", p=P))

                    # ---- gi1 matmul ----
                    with tc.high_priority():
                        for m in range(M3):
                            for k in range(K8):
                                nc.tensor.matmul(
                                    psA[:, m * B:(m + 1) * B],
                                    wih1_sb[:, k * SH + m * P:k * SH + (m + 1) * P],
                                    agA[:, k * B:(k + 1) * B],
                                    start=(k == 0), stop=(k == K8 - 1))

                    # ---- layer-1 combine ----
                    if t == 0:
                        gh1u = ghh1bc
                        bprev = h0bc
                    else:
                        gh1u = rp.tile([P, M3 * B], dt, tag="gh1u")
                        nc.vector.tensor_copy(gh1u[:], psC[:])
                        nc.vector.copy_predicated(gh1u[:], mru8_p[:], ghh1bc[:])
                        nc.vector.copy_predicated(gh1u[:], miu8_p[:], gh1u_prev[:])
                        bprev = bused_prev
                    gh1u_prev = gh1u

                    u_r = rp.tile([P, B], dt, tag="u_r")
                    u_z = rp.tile([P, B], dt, tag="u_z")
                    u_n = rp.tile([P, B], dt, tag="u_n")
                    braw = rp.tile([P, B], dt, tag="braw")
                    nc.vector.tensor_add(u_r[:], psA[:, 0:B], gh1u[:, 0:B])
                    nc.scalar.activation(u_r[:], u_r[:], Sig, bias=bsum1_sb[:, 0:1])
                    nc.vector.tensor_add(u_z[:], psA[:, B:2 * B], gh1u[:, B:2 * B])
                    nc.scalar.activation(u_z[:], u_z[:], Sig, bias=bsum1_sb[:, 1:2])
                    nc.vector.tensor_scalar_add(u_n[:], gh1u[:, 2 * B:3 * B],
                                                bhn_sb[:, 1:2])
                    nc.vector.tensor_mul(u_n[:], u_r[:], u_n[:])
                    nc.vector.tensor_add(u_n[:], u_n[:], psA[:, 2 * B:3 * B])
                    nc.scalar.activation(u_n[:], u_n[:], Tanh, bias=bsum1_sb[:, 2:3])
                    nc.vector.tensor_sub(braw[:], bprev[:], u_n[:])
                    nc.vector.tensor_mul(braw[:], braw[:], u_z[:])
                    nc.vector.tensor_add(braw[:], braw[:], u_n[:])

                    if DBG and t == 0:
                        nc.sync.dma_start(dbg_w[:], wih0_sb[:, 0:2 * SH])
                        nc.sync.dma_start(dbg_gi0[:], g0t[:])
                        nc.sync.dma_start(dbg_araw[:], araw[:])
                        nc.sync.dma_start(dbg_agA[:], agA[:])
                        dpsA = rp.tile([P, M3 * B], dt, tag="dpsA")
                        nc.vector.tensor_copy(dpsA[:], psA[:])
                        nc.sync.dma_start(dbg_psA[:], dpsA[:])
                        nc.sync.dma_start(dbg_braw[:], braw[:])

                    # ---- partial gate dot + B exchange ----
                    nc.tensor.matmul(pssm[0:B, 0:1], braw[:], v2s_sb[:],
                                     start=True, stop=True)
                    bex = rp.tile([P, B + 1], dt, tag="bex")
                    nc.vector.tensor_copy(bex[:, 0:B], braw[:])
                    nc.vector.tensor_copy(bex[0:B, B:B + 1], pssm[0:B, 0:1])

                    agBin = rdp.tile([P, B + 1], dt, tag="agBin")
                    agBout = rdp.tile([NC * P, B + 1], dt, tag="agBout",
                                      addr_space="Shared")
                    nc.sync.dma_start(agBin[:], bex[:])
                    nc.gpsimd.collective_compute(
                        "AllGather", Alu.bypass, replica_groups=rg,
                        ins=[agBin[:]], outs=[agBout[:]])
                    agB = rp.tile([P, NC * (B + 1)], dt, tag="agB")
                    nc.scalar.dma_start(
                        agB.rearrange("p (i b) -> p i b", b=B + 1),
                        agBout.rearrange("(i p) b -> p i b", p=P))

                    # ---- gh0_raw(t+1) ----
                    if t < T - 1:
                        for m in range(M3):
                            for k in range(K8):
                                nc.tensor.matmul(
                                    psB[:, m * B:(m + 1) * B],
                                    whh0_sb[:, k * SH + m * P:k * SH + (m + 1) * P],
                                    agA[:, k * B:(k + 1) * B],
                                    start=(k == 0), stop=(k == K8 - 1))

                    # ---- gate / masks ----
                    sc = rp.tile([B, 1], dt, tag="sc")
                    nc.vector.tensor_reduce(
                        sc[:],
                        agB.rearrange("p (i b) -> p i b", b=B + 1)[0:B, :, B:B + 1],
                        mybir.AxisListType.XY, Alu.add)
                    if DBG and t == 0:
                        nc.sync.dma_start(dbg_sc[:], sc[:])
                        nc.sync.dma_start(dbg_g1x[:], G1x_sb[:])
                    tnext = min(t + 1, T - 1)
                    lg = rp.tile([B, 1], dt, tag="lg")
                    nc.vector.tensor_add(lg[:], sc[:], G1x_sb[:, tnext:tnext + 1])
                    g_t = rp.tile([B, 1], dt, tag="g_t")
                    nc.scalar.activation(g_t[:], lg[:], Sig, bias=c0_sb[:])
                    # g_out = g + islast*(1-g)
                    gm1 = rp.tile([B, 1], dt, tag="gm1")
                    nc.vector.tensor_scalar(gm1[:], g_t[:], -1.0, 1.0,
                                            Alu.mult, Alu.add)
                    nc.vector.tensor_mul(gm1[:], gm1[:], islastc_sb[:, t:t + 1])
                    gout = rp.tile([B, 1], dt, tag="gout")
                    nc.vector.tensor_add(gout[:], g_t[:], gm1[:])
                    nc.vector.tensor_mul(gates_sb[:, t:t + 1], gout[:],
                                         validc_sb[:, t:t + 1])
                    # pack [reset | invalid | valid] cols -> rows -> bcasts
                    pack = rp.tile([B, 3], dt, tag="pack")
                    nc.vector.tensor_scalar(pack[:, 0:1], gout[:], 0.5, None,
                                            Alu.is_gt)
                    nc.vector.tensor_mul(pack[:, 0:1], pack[:, 0:1],
                                         validc_sb[:, t:t + 1])
                    nc.vector.tensor_copy(pack[:, 1:2], invalidc_sb[:, t:t + 1])
                    nc.vector.tensor_copy(pack[:, 2:3], validc_sb[:, t:t + 1])
                    with tc.high_priority():
                        for j in range(3):
                            nc.tensor.matmul(
                                pssm[0:1, 4 + j * B:4 + (j + 1) * B],
                                pack[:, j:j + 1], id64_sb[:],
                                start=True, stop=True)
                    rows = rp.tile([1, 3 * B], dt, tag="rows")
                    nc.vector.tensor_copy(rows[:], pssm[0:1, 4:4 + 3 * B])
                    with tc.high_priority():
                        for j in range(3):
                            dst = psmr if j == 0 else psmi
                            if j < 2:
                                for m in range(M3):
                                    nc.tensor.matmul(
                                        dst[:, m * B:(m + 1) * B], ones1_sb[:],
                                        rows[0:1, j * B:(j + 1) * B],
                                        start=True, stop=True)
                            else:
                                nc.tensor.matmul(
                                    pssm[:, 196:196 + B], ones1_sb[:],
                                    rows[0:1, 2 * B:3 * B], start=True, stop=True)

                    maskr_u8 = rp.tile([P, M3 * B], mybir.dt.uint8, tag="mru8")
                    maski_u8 = rp.tile([P, M3 * B], mybir.dt.uint8, tag="miu8")
                    nc.vector.tensor_copy(maskr_u8[:], psmr[:])
                    nc.vector.tensor_copy(maski_u8[:], psmi[:])
                    maskr_u8_prev, maski_u8_prev = maskr_u8, maski_u8

                    # ---- masked state upkeep ----
                    aused = rp.tile([P, B], dt, tag="aused")
                    nc.vector.tensor_copy(aused[:], araw[:])
                    nc.vector.copy_predicated(aused[:], maskr_u8[:, 0:B], h0bc[:])
                    nc.vector.copy_predicated(aused[:], maski_u8[:, 0:B], aprev[:])
                    bused = rp.tile([P, B], dt, tag="bused")
                    nc.vector.tensor_copy(bused[:], braw[:])
                    nc.vector.copy_predicated(bused[:], maskr_u8[:, 0:B], h0bc[:])
                    nc.vector.copy_predicated(bused[:], maski_u8[:, 0:B], bprev[:])
                    aused_prev, bused_prev = aused, bused

                    # ---- masked output ----
                    outm = rp.tile([P, B], dt, tag="outm", bufs=3)
                    nc.vector.tensor_mul(outm[:], braw[:], pssm[:, 196:196 + B])
                    nc.gpsimd.dma_start(out_d[t], outm[:])

                    # ---- gh1_raw(t+1) ----
                    if t < T - 1:
                        agB_v = agB.rearrange("p (i b) -> p i b", b=B + 1)
                        for m in range(M3):
                            for k in range(K8):
                                nc.tensor.matmul(
                                    psC[:, m * B:(m + 1) * B],
                                    whh1_sb[:, k * SH + m * P:k * SH + (m + 1) * P],
                                    agB_v[:, k, 0:B],
                                    start=(k == 0), stop=(k == K8 - 1))

                    mru8_p, miu8_p = maskr_u8, maski_u8

                nc.sync.dma_start(gates_d[:], gates_sb[:])

    nc.compile()
    return nc


def kernel(x, h0, input_lengths, W_ih, W_hh, b_ih, b_hh,
           gW1, gb1, gW2, gb2, gWF, gbF):
    x = np.asarray(x, np.float32)
    h0 = np.asarray(h0, np.float32)
    lengths = np.asarray(input_lengths, np.int32)
    W_ih = np.asarray(W_ih, np.float32)
    W_hh = np.asarray(W_hh, np.float32)
    b_ih = np.asarray(b_ih, np.float32)
    b_hh = np.asarray(b_hh, np.float32)
    gW1 = np.asarray(gW1, np.float32)
    gb1 = np.asarray(gb1, np.float32)
    gW2 = np.asarray(gW2, np.float32)
    gb2 = np.asarray(gb2, np.float32)
    gWF = np.asarray(gWF, np.float32)
    gbF = np.asarray(gbF, np.float32)

    Bx, T, Hx = x.shape
    assert Bx == B and Hx == H
    TCH = T * B // NC

    if T not in _prog_cache:
        _prog_cache[T] = _build(T)
    nc = _prog_cache[T]

    # ---------------- host prep ----------------
    v1 = (gWF @ gW1)[0]                      # [H]
    v2 = (gWF @ gW2)[0]
    c0 = np.float32((gWF @ (gb1 + gb2) + gbF)[0])
    bsum0 = b_ih[0] + b_hh[0]
    bsum0[2 * H:3 * H] = b_ih[0][2 * H:3 * H]          # n-block: b_ih only
    bsum1 = b_ih[1] + b_hh[1]
    bsum1[2 * H:3 * H] = b_ih[1][2 * H:3 * H]

    # xT with col = t*B + b
    xT = np.ascontiguousarray(x.transpose(2, 1, 0).reshape(H, T * B))

    ts = np.arange(T)[None, :]
    valid = (ts < lengths[:, None]).astype(np.float32)           # [B,T]
    islast = (ts == (lengths[:, None] - 1)).astype(np.float32)
    invalid = 1.0 - valid

    id64 = np.eye(B, dtype=np.float32)
    ones1 = np.ones((1, P), np.float32)
    h0T = np.ascontiguousarray(h0.reshape(K8, P).T)              # [P, K8]

    in_maps = []
    for c in range(NC):
        hs = slice(c * P, (c + 1) * P)

        def shard(W):
            cols = [np.ascontiguousarray(
                W[g * H + c * P:g * H + (c + 1) * P, :].T) for g in range(3)]
            return np.ascontiguousarray(np.concatenate(cols, axis=1))  # [H, SH]

        def bshard(bvec):
            return np.stack([bvec[g * H + c * P:g * H + (c + 1) * P]
                             for g in range(3)], axis=1).astype(np.float32)

        in_maps.append({
            "xT": np.ascontiguousarray(xT[:, c * TCH:(c + 1) * TCH]),
            "wih0": shard(W_ih[0]),
            "whh0": shard(W_hh[0]),
            "wih1": shard(W_ih[1]),
            "whh1": shard(W_hh[1]),
            "bsum0": bshard(bsum0),
            "bsum1": bshard(bsum1),
            "v1": np.ascontiguousarray(v1.reshape(K8, P).T),
            "v2s": np.ascontiguousarray(v2[hs, None]),
            "h0T": h0T,
            "h0c": np.ascontiguousarray(h0[0, hs, None]),
            "validc": valid,
            "invalidc": invalid,
            "islastc": islast,
            "id64": id64,
            "ones1": ones1,
            "c0": np.full((B, 1), c0, np.float32),
            "bhn": np.stack([b_hh[0][2 * H + c * P:2 * H + (c + 1) * P],
                             b_hh[1][2 * H + c * P:2 * H + (c + 1) * P]],
                            axis=1).astype(np.float32),
        })

    trace = os.environ.get("KERNEL_TRACE", "0") == "1"
    res = run_bass_kernel_spmd(nc, in_maps, core_ids=list(range(NC)), trace=trace)
    LAST_EXEC_NS[0] = res.exec_time_ns

    # ---------------- unshard ----------------
    # core c out: [T, P, B] holding h-slice c; full[b, t, c*P + p] = out_c[t, p, b]
    outs = [res.results[c]["out"] for c in range(NC)]
    stacked = np.stack(outs, axis=0)                  # [NC, T, P, B]
    output = stacked.transpose(3, 1, 0, 2).reshape(B, T, H)
    gate_z = res.results[0]["gates"]                  # [B, T]
    return np.ascontiguousarray(output), np.ascontiguousarray(gate_z)


# revision 18
# speedup vs baseline: 1.1201x; 1.1201x over previous
"""AdaptiveGRU (2-layer, H=1024, adaptive reset gate) on 8 Trainium2 NeuronCores.

Strategy: tensor-parallel over the 3H gate dim (each core owns a 128-row
h-slice of both layers), fp32 everywhere.
 - Layer-0 input projections (x @ W_ih0.T + biases) are precomputed for all
   timesteps as one large GEMM (x distributed to all cores via a startup
   AllGather of host-transposed shards).
 - The BinaryGate is folded: g = sigmoid(x_{t+1}.v1 + out_t.v2 + c) with
   v1 = (gWF@gW1), v2 = (gWF@gW2), c = gWF.(gb1+gb2)+gbF.
 - Per step: 2 AllGathers (a_raw slices; b_raw slices + partial gate dots).
 - Reset/freeze masking is applied on matmul OUTPUTS (select-on-output):
   gh_used = select(reset -> W@h0, invalid -> prev used, else W@state_raw),
   which keeps masking off the matmul critical path.
"""
import os
import sys

sys.path.insert(0, '/opt/trn_rl_repo')

import numpy as np

import concourse.bass as bass
import concourse.bacc as bacc
import concourse.mybir as mybir
from concourse import tile
from concourse.bass_utils import run_bass_kernel_spmd

H = 1024
B = 64
NC = 8
P = 128
SH = 3 * H // NC       # 384 rows of each gate matrix per core
M3 = 3                 # m-tiles (r, z, n) per core
K8 = H // P            # 8 contraction tiles
dt = mybir.dt.float32

LAST_EXEC_NS = [None]

_prog_cache = {}


def _build(T):
    TB = T * B
    TCH = TB // NC                       # per-core xT chunk cols
    CHUNK = min(512, TCH)                # precompute N-chunk size
    NCH = TB // CHUNK
    CH_PER_BLK = TCH // CHUNK            # chunks per AG block

    nc = bacc.Bacc("TRN2", target_bir_lowering=False, debug=False, num_devices=NC)

    # ---------------- DRAM I/O ----------------
    xT_d = nc.dram_tensor("xT", [H, TCH], dt, kind="ExternalInput")
    wih0_d = nc.dram_tensor("wih0", [H, SH], dt, kind="ExternalInput")
    whh0_d = nc.dram_tensor("whh0", [H, SH], dt, kind="ExternalInput")
    wih1_d = nc.dram_tensor("wih1", [H, SH], dt, kind="ExternalInput")
    whh1_d = nc.dram_tensor("whh1", [H, SH], dt, kind="ExternalInput")
    bsum0_d = nc.dram_tensor("bsum0", [P, 3], dt, kind="ExternalInput")
    bsum1_d = nc.dram_tensor("bsum1", [P, 3], dt, kind="ExternalInput")
    v1_d = nc.dram_tensor("v1", [P, K8], dt, kind="ExternalInput")
    v2s_d = nc.dram_tensor("v2s", [P, 1], dt, kind="ExternalInput")
    h0T_d = nc.dram_tensor("h0T", [P, K8], dt, kind="ExternalInput")
    h0c_d = nc.dram_tensor("h0c", [P, 1], dt, kind="ExternalInput")
    validc_d = nc.dram_tensor("validc", [B, T], dt, kind="ExternalInput")
    invalidc_d = nc.dram_tensor("invalidc", [B, T], dt, kind="ExternalInput")
    islastc_d = nc.dram_tensor("islastc", [B, T], dt, kind="ExternalInput")
    id64_d = nc.dram_tensor("id64", [B, B], dt, kind="ExternalInput")
    ones1_d = nc.dram_tensor("ones1", [1, P], dt, kind="ExternalInput")
    c0_d = nc.dram_tensor("c0", [B, 1], dt, kind="ExternalInput")
    bhn_d = nc.dram_tensor("bhn", [P, 2], dt, kind="ExternalInput")

    out_d = nc.dram_tensor("out", [T, P, B], dt, kind="ExternalOutput")
    gates_d = nc.dram_tensor("gates", [B, T], dt, kind="ExternalOutput")
    DBG = os.environ.get("KERNEL_DEBUG", "0") == "1"
    if DBG:
        dbg_gi0 = nc.dram_tensor("dbg_gi0", [P, M3 * B], dt, kind="ExternalOutput")
        dbg_araw = nc.dram_tensor("dbg_araw", [P, B], dt, kind="ExternalOutput")
        dbg_agA = nc.dram_tensor("dbg_agA", [P, NC * B], dt, kind="ExternalOutput")
        dbg_psA = nc.dram_tensor("dbg_psA", [P, M3 * B], dt, kind="ExternalOutput")
        dbg_braw = nc.dram_tensor("dbg_braw", [P, B], dt, kind="ExternalOutput")
        dbg_sc = nc.dram_tensor("dbg_sc", [B, 1], dt, kind="ExternalOutput")
        dbg_g1x = nc.dram_tensor("dbg_g1x", [B, T], dt, kind="ExternalOutput")
        dbg_w = nc.dram_tensor("dbg_w", [P, 2 * SH], dt, kind="ExternalOutput")
        dbg_ev = nc.dram_tensor("dbg_ev", [P, 512], dt, kind="ExternalOutput")
        dbg_xt = nc.dram_tensor("dbg_xt", [P, 512], dt, kind="ExternalOutput")

    rg = [list(range(NC))]
    Sig = mybir.ActivationFunctionType.Sigmoid
    Tanh = mybir.ActivationFunctionType.Tanh
    Alu = mybir.AluOpType

    with tile.TileContext(nc) as tc:
        with (
            tc.tile_pool(name="const", bufs=1) as cp,
            tc.tile_pool(name="dram", bufs=1, space="DRAM") as dp,
        ):
            # ---------- resident constants ----------
            wih0_sb = cp.tile([P, K8 * SH], dt)
            whh0_sb = cp.tile([P, K8 * SH], dt)
            wih1_sb = cp.tile([P, K8 * SH], dt)
            whh1_sb = cp.tile([P, K8 * SH], dt)
            for w_sb, w_d in ((wih0_sb, wih0_d), (whh0_sb, whh0_d),
                              (wih1_sb, wih1_d), (whh1_sb, whh1_d)):
                nc.sync.dma_start(
                    w_sb.rearrange("p (k m) -> p k m", m=SH),
                    w_d.ap().rearrange("(k p) m -> p k m", p=P))

            bsum0_sb = cp.tile([P, 3], dt)
            bsum1_sb = cp.tile([P, 3], dt)
            v1_sb = cp.tile([P, K8], dt)
            v2s_sb = cp.tile([P, 1], dt)
            h0T_sb = cp.tile([P, K8], dt)
            h0c_sb = cp.tile([P, 1], dt)
            validc_sb = cp.tile([B, T], dt)
            invalidc_sb = cp.tile([B, T], dt)
            islastc_sb = cp.tile([B, T], dt)
            id64_sb = cp.tile([B, B], dt)
            ones1_sb = cp.tile([1, P], dt)
            c0_sb = cp.tile([B, 1], dt)
            bhn_sb = cp.tile([P, 2], dt)
            for sb, d in ((bsum0_sb, bsum0_d), (bsum1_sb, bsum1_d),
                          (v1_sb, v1_d), (v2s_sb, v2s_d), (h0T_sb, h0T_d),
                          (h0c_sb, h0c_d), (validc_sb, validc_d),
                          (invalidc_sb, invalidc_d), (islastc_sb, islastc_d),
                          (id64_sb, id64_d), (ones1_sb, ones1_d), (c0_sb, c0_d),
                          (bhn_sb, bhn_d)):
                nc.sync.dma_start(sb[:], d[:])

            ones_sb = cp.tile([P, B], dt)
            nc.vector.memset(ones_sb[:], 1.0)
            zeros_sb = cp.tile([P, B], dt)
            nc.vector.memset(zeros_sb[:], 0.0)
            h0bc = cp.tile([P, B], dt)
            nc.vector.tensor_scalar_mul(h0bc[:], ones_sb[:], h0c_sb[:, 0:1])

            ghh0bc = cp.tile([P, M3 * B], dt)   # broadcast W_hh0_shard @ h0
            ghh1bc = cp.tile([P, M3 * B], dt)
            G1x_sb = cp.tile([B, T], dt)        # x_{.}.v1 columns
            gates_sb = cp.tile([B, T], dt)

            # internal DRAM
            xt_int = dp.tile([H, TCH], dt, name="xt_int")
            xt_full = dp.tile([NC * H, TCH], dt, name="xt_full", addr_space="Shared")
            gi0_d = dp.tile([M3, P, TB], dt, name="gi0_d")

            # =======================================================
            # Phase 0: distribute xT
            # =======================================================
            nc.sync.dma_start(xt_int[:], xT_d[:])
            nc.gpsimd.collective_compute(
                "AllGather", Alu.bypass, replica_groups=rg,
                ins=[xt_int[:]], outs=[xt_full[:]])

            # =======================================================
            # Phase 1: precompute gi0 = x@Wih0_shard.T + bsum0, G1x = x.v1,
            #          ghh0/ghh1 consts
            # =======================================================
            with (
                tc.tile_pool(name="pre_sb", bufs=1) as pp,
                tc.tile_pool(name="pre_ps", bufs=1, space="PSUM") as pps,
            ):
                # ghh consts: W_shard @ h0  -> [P, 3] -> broadcast
                psh = pps.tile([P, 4], dt, tag="psh")
                ghc = pp.tile([P, 3], dt, tag="ghc")
                for w_sb, bc in ((whh0_sb, ghh0bc), (whh1_sb, ghh1bc)):
                    for m in range(M3):
                        for k in range(K8):
                            nc.tensor.matmul(
                                psh[:, m:m + 1],
                                w_sb[:, k * SH + m * P:k * SH + (m + 1) * P],
                                h0T_sb[:, k:k + 1],
                                start=(k == 0), stop=(k == K8 - 1))
                    nc.vector.tensor_copy(ghc[:], psh[:, 0:3])
                    for m in range(M3):
                        nc.vector.tensor_scalar_mul(
                            bc[:, m * B:(m + 1) * B], ones_sb[:], ghc[:, m:m + 1])

                for n in range(NCH):
                    blk, coff = divmod(n, CH_PER_BLK)
                    xt = pp.tile([P, K8 * CHUNK], dt, tag="xt", bufs=3)
                    nc.sync.dma_start(
                        xt.rearrange("p (k c) -> p k c", c=CHUNK),
                        xt_full[blk * H:(blk + 1) * H,
                                coff * CHUNK:(coff + 1) * CHUNK]
                        .rearrange("(k p) c -> p k c", p=P))
                    for m in range(M3):
                        ps = pps.tile([P, CHUNK], dt, tag="psPC", bufs=2)
                        for k in range(K8):
                            nc.tensor.matmul(
                                ps[:],
                                wih0_sb[:, k * SH + m * P:k * SH + (m + 1) * P],
                                xt[:, k * CHUNK:(k + 1) * CHUNK],
                                start=(k == 0), stop=(k == K8 - 1))
                        ev = pp.tile([P, CHUNK], dt, tag="ev", bufs=3)
                        nc.vector.tensor_scalar_add(ev[:], ps[:], bsum0_sb[:, m:m + 1])
                        if DBG and n == 0 and m == 0:
                            nc.sync.dma_start(dbg_ev[:, 0:CHUNK], ev[:])
                            nc.sync.dma_start(dbg_xt[:, 0:CHUNK], xt[:, 0:CHUNK])
                        nc.sync.dma_start(gi0_d[m, :, n * CHUNK:(n + 1) * CHUNK], ev[:])
                    for j in range(CHUNK // B):
                        tau = n * (CHUNK // B) + j
                        psg = pps.tile([B, 1], dt, tag="psg", bufs=2)
                        for k in range(K8):
                            nc.tensor.matmul(
                                psg[:],
                                xt[:, k * CHUNK + j * B:k * CHUNK + (j + 1) * B],
                                v1_sb[:, k:k + 1],
                                start=(k == 0), stop=(k == K8 - 1))
                        nc.vector.tensor_copy(G1x_sb[:, tau:tau + 1], psg[:])


            # =======================================================
            # Phase 2: recurrence
            # =======================================================
            with (
                tc.tile_pool(name="rec_sb", bufs=2) as rp,
                tc.tile_pool(name="rec_ps", bufs=1, space="PSUM") as rps,
                tc.tile_pool(name="rec_dram", bufs=2, space="DRAM") as rdp,
            ):
                psA = rps.tile([P, M3 * B], dt, tag="psA")
                psB = rps.tile([P, M3 * B], dt, tag="psB")
                psC = rps.tile([P, M3 * B], dt, tag="psC")
                psmr = rps.tile([P, M3 * B], dt, tag="psmr")   # reset bcast x3
                psmi = rps.tile([P, M3 * B], dt, tag="psmi")   # invalid bcast x3
                pssm = rps.tile([P, 260], dt, tag="pssm")      # pdot / rows3 / validbc

                gh0u_prev = None
                gh1u_prev = None
                mru8_p = None
                miu8_p = None
                aused_prev = None
                bused_prev = None
                agA_prev = None
                agB_prev = None

                for t in range(T):
                    # ---- layer-0 combine ----
                    g0t = rp.tile([P, M3 * B], dt, tag="g0t", bufs=3)
                    nc.sync.dma_start(
                        g0t.rearrange("p (m c) -> p m c", c=B),
                        gi0_d[:, :, t * B:(t + 1) * B].rearrange("m p c -> p m c"))

                    if t == 0:
                        gh0u = ghh0bc
                        aprev = h0bc
                    else:
                        gh0u = rp.tile([P, M3 * B], dt, tag="gh0u")
                        nc.vector.tensor_copy(gh0u[:], psB[:])
                        nc.vector.copy_predicated(gh0u[:], mru8_p[:], ghh0bc[:])
                        nc.vector.copy_predicated(gh0u[:], miu8_p[:], gh0u_prev[:])
                        aprev = aused_prev
                    gh0u_prev = gh0u

                    t_r = rp.tile([P, B], dt, tag="t_r")
                    t_z = rp.tile([P, B], dt, tag="t_z")
                    t_n = rp.tile([P, B], dt, tag="t_n")
                    araw = rp.tile([P, B], dt, tag="araw")
                    nc.vector.tensor_add(t_r[:], g0t[:, 0:B], gh0u[:, 0:B])
                    nc.scalar.activation(t_r[:], t_r[:], Sig)
                    nc.vector.tensor_add(t_z[:], g0t[:, B:2 * B], gh0u[:, B:2 * B])
                    nc.scalar.activation(t_z[:], t_z[:], Sig)
                    nc.vector.tensor_scalar_add(t_n[:], gh0u[:, 2 * B:3 * B],
                                                bhn_sb[:, 0:1])
                    nc.vector.tensor_mul(t_n[:], t_r[:], t_n[:])
                    nc.vector.tensor_add(t_n[:], t_n[:], g0t[:, 2 * B:3 * B])
                    nc.scalar.activation(t_n[:], t_n[:], Tanh)
                    # a_raw = n + z*(aprev - n)
                    nc.vector.tensor_sub(araw[:], aprev[:], t_n[:])
                    nc.vector.tensor_mul(araw[:], araw[:], t_z[:])
                    nc.vector.tensor_add(araw[:], araw[:], t_n[:])

                    # ---- A exchange ----
                    agAin = rdp.tile([P, B], dt, tag="agAin")
                    agAout = rdp.tile([NC * P, B], dt, tag="agAout",
                                      addr_space="Shared")
                    nc.sync.dma_start(agAin[:], araw[:])
                    nc.gpsimd.collective_compute(
                        "AllGather", Alu.bypass, replica_groups=rg,
                        ins=[agAin[:]], outs=[agAout[:]])
                    agA = rp.tile([P, NC * B], dt, tag="agA")
                    nc.sync.dma_start(
                        agA.rearrange("p (i b) -> p i b", b=B),
                        agAout.rearrange("(i p) b -> p i b", p=P))

                    # ---- gi1 matmul ----
                    with tc.high_priority():
                        for m in range(M3):
                            for k in range(K8):
                                nc.tensor.matmul(
                                    psA[:, m * B:(m + 1) * B],
                                    wih1_sb[:, k * SH + m * P:k * SH + (m + 1) * P],
                                    agA[:, k * B:(k + 1) * B],
                                    start=(k == 0), stop=(k == K8 - 1))

                    # ---- layer-1 combine ----
                    if t == 0:
                        gh1u = ghh1bc
                        bprev = h0bc
                    else:
                        gh1u = rp.tile([P, M3 * B], dt, tag="gh1u")
                        nc.vector.tensor_copy(gh1u[:], psC[:])
                        nc.vector.copy_predicated(gh1u[:], mru8_p[:], ghh1bc[:])
                        nc.vector.copy_predicated(gh1u[:], miu8_p[:], gh1u_prev[:])
                        bprev = bused_prev
                    gh1u_prev = gh1u

                    u_r = rp.tile([P, B], dt, tag="u_r")
                    u_z = rp.tile([P, B], dt, tag="u_z")
                    u_n = rp.tile([P, B], dt, tag="u_n")
                    braw = rp.tile([P, B], dt, tag="braw")
                    nc.vector.tensor_add(u_r[:], psA[:, 0:B], gh1u[:, 0:B])
                    nc.scalar.activation(u_r[:], u_r[:], Sig, bias=bsum1_sb[:, 0:1])
                    nc.vector.tensor_add(u_z[:], psA[:, B:2 * B], gh1u[:, B:2 * B])
                    nc.scalar.activation(u_z[:], u_z[:], Sig, bias=bsum1_sb[:, 1:2])
                    nc.vector.tensor_scalar_add(u_n[:], gh1u[:, 2 * B:3 * B],
                                                bhn_sb[:, 1:2])
                    nc.vector.tensor_mul(u_n[:], u_r[:], u_n[:])
                    nc.vector.tensor_add(u_n[:], u_n[:], psA[:, 2 * B:3 * B])
                    nc.scalar.activation(u_n[:], u_n[:], Tanh, bias=bsum1_sb[:, 2:3])
                    nc.vector.tensor_sub(braw[:], bprev[:], u_n[:])
                    nc.vector.tensor_mul(braw[:], braw[:], u_z[:])
                    nc.vector.tensor_add(braw[:], braw[:], u_n[:])

                    if DBG and t == 0:
                        nc.sync.dma_start(dbg_w[:], wih0_sb[:, 0:2 * SH])
                        nc.sync.dma_start(dbg_gi0[:], g0t[:])
                        nc.sync.dma_start(dbg_araw[:], araw[:])
                        nc.sync.dma_start(dbg_agA[:], agA[:])
                        dpsA = rp.tile([P, M3 * B], dt, tag="dpsA")
                        nc.vector.tensor_copy(dpsA[:], psA[:])
                        nc.sync.dma_start(dbg_psA[:], dpsA[:])
                        nc.sync.dma_start(dbg_braw[:], braw[:])

                    # ---- partial gate dot + B exchange ----
                    nc.tensor.matmul(pssm[0:B, 0:1], braw[:], v2s_sb[:],
                                     start=True, stop=True)
                    bex = rp.tile([P, B + 1], dt, tag="bex")
                    nc.vector.tensor_copy(bex[:, 0:B], braw[:])
                    nc.vector.tensor_copy(bex[0:B, B:B + 1], pssm[0:B, 0:1])

                    agBin = rdp.tile([P, B + 1], dt, tag="agBin")
                    agBout = rdp.tile([NC * P, B + 1], dt, tag="agBout",
                                      addr_space="Shared")
                    nc.sync.dma_start(agBin[:], bex[:])
                    nc.gpsimd.collective_compute(
                        "AllGather", Alu.bypass, replica_groups=rg,
                        ins=[agBin[:]], outs=[agBout[:]])
                    agB = rp.tile([P, NC * (B + 1)], dt, tag="agB")
                    nc.sync.dma_start(
                        agB.rearrange("p (i b) -> p i b", b=B + 1),
                        agBout.rearrange("(i p) b -> p i b", p=P))

                    # ---- gh0_raw(t+1) ----
                    if t < T - 1:
                        for m in range(M3):
                            for k in range(K8):
                                nc.tensor.matmul(
                                    psB[:, m * B:(m + 1) * B],
                                    whh0_sb[:, k * SH + m * P:k * SH + (m + 1) * P],
                                    agA[:, k * B:(k + 1) * B],
                                    start=(k == 0), stop=(k == K8 - 1))

                    # ---- gate / masks ----
                    sc = rp.tile([B, 1], dt, tag="sc")
                    nc.vector.tensor_reduce(
                        sc[:],
                        agB.rearrange("p (i b) -> p i b", b=B + 1)[0:B, :, B:B + 1],
                        mybir.AxisListType.XY, Alu.add)
                    if DBG and t == 0:
                        nc.sync.dma_start(dbg_sc[:], sc[:])
                        nc.sync.dma_start(dbg_g1x[:], G1x_sb[:])
                    tnext = min(t + 1, T - 1)
                    lg = rp.tile([B, 1], dt, tag="lg")
                    nc.vector.tensor_add(lg[:], sc[:], G1x_sb[:, tnext:tnext + 1])
                    g_t = rp.tile([B, 1], dt, tag="g_t")
                    nc.scalar.activation(g_t[:], lg[:], Sig, bias=c0_sb[:])
                    # g_out = g + islast*(1-g)
                    gm1 = rp.tile([B, 1], dt, tag="gm1")
                    nc.vector.tensor_scalar(gm1[:], g_t[:], -1.0, 1.0,
                                            Alu.mult, Alu.add)
                    nc.vector.tensor_mul(gm1[:], gm1[:], islastc_sb[:, t:t + 1])
                    gout = rp.tile([B, 1], dt, tag="gout")
                    nc.vector.tensor_add(gout[:], g_t[:], gm1[:])
                    nc.vector.tensor_mul(gates_sb[:, t:t + 1], gout[:],
                                         validc_sb[:, t:t + 1])
                    # pack [reset | invalid | valid] cols -> rows -> bcasts
                    pack = rp.tile([B, 3], dt, tag="pack")
                    nc.vector.tensor_scalar(pack[:, 0:1], gout[:], 0.5, None,
                                            Alu.is_gt)
                    nc.vector.tensor_mul(pack[:, 0:1], pack[:, 0:1],
                                         validc_sb[:, t:t + 1])
                    nc.vector.tensor_copy(pack[:, 1:2], invalidc_sb[:, t:t + 1])
                    nc.vector.tensor_copy(pack[:, 2:3], validc_sb[:, t:t + 1])
                    with tc.high_priority():
                        for j in range(3):
                            nc.tensor.matmul(
                                pssm[0:1, 4 + j * B:4 + (j + 1) * B],
                                pack[:, j:j + 1], id64_sb[:],
                                start=True, stop=True)
                    rows = rp.tile([1, 3 * B], dt, tag="rows")
                    nc.vector.tensor_copy(rows[:], pssm[0:1, 4:4 + 3 * B])
                    with tc.high_priority():
                        for j in range(3):
                            dst = psmr if j == 0 else psmi
                            if j < 2:
                                for m in range(M3):
                                    nc.tensor.matmul(
                                        dst[:, m * B:(m + 1) * B], ones1_sb[:],
                                        rows[0:1, j * B:(j + 1) * B],
                                        start=True, stop=True)
                            else:
                                nc.tensor.matmul(
                                    pssm[:, 196:196 + B], ones1_sb[:],
                                    rows[0:1, 2 * B:3 * B], start=True, stop=True)

                    maskr_u8 = rp.tile([P, M3 * B], mybir.dt.uint8, tag="mru8")
                    maski_u8 = rp.tile([P, M3 * B], mybir.dt.uint8, tag="miu8")
                    nc.vector.tensor_copy(maskr_u8[:], psmr[:])
                    nc.vector.tensor_copy(maski_u8[:], psmi[:])
                    maskr_u8_prev, maski_u8_prev = maskr_u8, maski_u8

                    # ---- masked state upkeep ----
                    aused = rp.tile([P, B], dt, tag="aused")
                    nc.vector.tensor_copy(aused[:], araw[:])
                    nc.vector.copy_predicated(aused[:], maskr_u8[:, 0:B], h0bc[:])
                    nc.vector.copy_predicated(aused[:], maski_u8[:, 0:B], aprev[:])
                    bused = rp.tile([P, B], dt, tag="bused")
                    nc.vector.tensor_copy(bused[:], braw[:])
                    nc.vector.copy_predicated(bused[:], maskr_u8[:, 0:B], h0bc[:])
                    nc.vector.copy_predicated(bused[:], maski_u8[:, 0:B], bprev[:])
                    aused_prev, bused_prev = aused, bused

                    # ---- masked output ----
                    outm = rp.tile([P, B], dt, tag="outm", bufs=3)
                    nc.vector.tensor_mul(outm[:], braw[:], pssm[:, 196:196 + B])
                    nc.sync.dma_start(out_d[t], outm[:])

                    # ---- gh1_raw(t+1) ----
                    if t < T - 1:
                        agB_v = agB.rearrange("p (i b) -> p i b", b=B + 1)
                        for m in range(M3):
                            for k in range(K8):
                                nc.tensor.matmul(
                                    psC[:, m * B:(m + 1) * B],
                                    whh1_sb[:, k * SH + m * P:k * SH + (m + 1) * P],
                                    agB_v[:, k, 0:B],
                                    start=(k == 0), stop=(k == K8 - 1))

                    mru8_p, miu8_p = maskr_u8, maski_u8

                nc.sync.dma_start(gates_d[:], gates_sb[:])

    nc.compile()
    return nc


def kernel(x, h0, input_lengths, W_ih, W_hh, b_ih, b_hh,
           gW1, gb1, gW2, gb2, gWF, gbF):
    x = np.asarray(x, np.float32)
    h0 = np.asarray(h0, np.float32)
    lengths = np.asarray(input_lengths, np.int32)
    W_ih = np.asarray(W_ih, np.float32)
    W_hh = np.asarray(W_hh, np.float32)
    b_ih = np.asarray(b_ih, np.float32)
    b_hh = np.asarray(b_hh, np.float32)
    gW1 = np.asarray(gW1, np.float32)
    gb1 = np.asarray(gb1, np.float32)
    gW2 = np.asarray(gW2, np.float32)
    gb2 = np.asarray(gb2, np.float32)
    gWF = np.asarray(gWF, np.float32)
    gbF = np.asarray(gbF, np.float32)

    Bx, T, Hx = x.shape
    assert Bx == B and Hx == H
    TCH = T * B // NC

    if T not in _prog_cache:
        _prog_cache[T] = _build(T)
    nc = _prog_cache[T]

    # ---------------- host prep ----------------
    v1 = (gWF @ gW1)[0]                      # [H]
    v2 = (gWF @ gW2)[0]
    c0 = np.float32((gWF @ (gb1 + gb2) + gbF)[0])
    bsum0 = b_ih[0] + b_hh[0]
    bsum0[2 * H:3 * H] = b_ih[0][2 * H:3 * H]          # n-block: b_ih only
    bsum1 = b_ih[1] + b_hh[1]
    bsum1[2 * H:3 * H] = b_ih[1][2 * H:3 * H]

    # xT with col = t*B + b
    xT = np.ascontiguousarray(x.transpose(2, 1, 0).reshape(H, T * B))

    ts = np.arange(T)[None, :]
    valid = (ts < lengths[:, None]).astype(np.float32)           # [B,T]
    islast = (ts == (lengths[:, None] - 1)).astype(np.float32)
    invalid = 1.0 - valid

    id64 = np.eye(B, dtype=np.float32)
    ones1 = np.ones((1, P), np.float32)
    h0T = np.ascontiguousarray(h0.reshape(K8, P).T)              # [P, K8]

    in_maps = []
    for c in range(NC):
        hs = slice(c * P, (c + 1) * P)

        def shard(W):
            cols = [np.ascontiguousarray(
                W[g * H + c * P:g * H + (c + 1) * P, :].T) for g in range(3)]
            return np.ascontiguousarray(np.concatenate(cols, axis=1))  # [H, SH]

        def bshard(bvec):
            return np.stack([bvec[g * H + c * P:g * H + (c + 1) * P]
                             for g in range(3)], axis=1).astype(np.float32)

        in_maps.append({
            "xT": np.ascontiguousarray(xT[:, c * TCH:(c + 1) * TCH]),
            "wih0": shard(W_ih[0]),
            "whh0": shard(W_hh[0]),
            "wih1": shard(W_ih[1]),
            "whh1": shard(W_hh[1]),
            "bsum0": bshard(bsum0),
            "bsum1": bshard(bsum1),
            "v1": np.ascontiguousarray(v1.reshape(K8, P).T),
            "v2s": np.ascontiguousarray(v2[hs, None]),
            "h0T": h0T,
            "h0c": np.ascontiguousarray(h0[0, hs, None]),
            "validc": valid,
            "invalidc": invalid,
            "islastc": islast,
            "id64": id64,
            "ones1": ones1,
            "c0": np.full((B, 1), c0, np.float32),
            "bhn": np.stack([b_hh[0][2 * H + c * P:2 * H + (c + 1) * P],
                             b_hh[1][2 * H + c * P:2 * H + (c + 1) * P]],
                            axis=1).astype(np.float32),
        })

    trace = os.environ.get("KERNEL_TRACE", "0") == "1"
    res = run_bass_kernel_spmd(nc, in_maps, core_ids=list(range(NC)), trace=trace)
    LAST_EXEC_NS[0] = res.exec_time_ns

    # ---------------- unshard ----------------
    # core c out: [T, P, B] holding h-slice c; full[b, t, c*P + p] = out_c[t, p, b]
    outs = [res.results[c]["out"] for c in range(NC)]
    stacked = np.stack(outs, axis=0)                  # [NC, T, P, B]
    output = stacked.transpose(3, 1, 0, 2).reshape(B, T, H)
    gate_z = res.results[0]["gates"]                  # [B, T]
    return np.ascontiguousarray(output), np.ascontiguousarray(gate_z)
